# revision 1
# baseline (speedup 1.0000x reference)
"""Trainium2 Bass kernel for the Dedicom decoder problem.

Math: with U = z * d (row-wise scale by the selected local_diag row),
    score_b = ((z[e0]*d) @ W) * d . z[e1] = U[e0] @ W @ U[e1]^T
so all-pairs scores S = (U @ W) @ U^T  ([N_DRUGS, N_DRUGS]) contain every
edge score.  We shard S by e0-block across the 8 cores: core c computes
S rows [512c, 512c+512) (~2.1 GF in bf16), streams them to DRAM, then a
256B-granular dma_gather pulls each edge's 128-wide candidate block and a
host-built one-hot mask + segmented reduce extracts the scalar, followed
by an on-chip sigmoid.  Edges are bucketed to cores by e0>>9 on the host;
results are scattered back to their original positions on the host.
"""

import numpy as np
import ml_dtypes

BF = ml_dtypes.bfloat16

N_DRUGS = 4096
D = 512
N_CORES = 8
BLK = N_DRUGS // N_CORES  # 512 rows of S per core
KC = D // 128             # 4 contraction chunks
MT = BLK // 128           # 4 row tiles of the core's S block
NCH = N_DRUGS // 512      # 8 column chunks of S
TPB = N_DRUGS // 128      # 32 tokens (128-wide blocks) per S row

_cache = {}


def _build(cap, dep_mode="helper", inplace=True, tail=True, gather_mode="real",
           ms_load=True):
    """Build + compile the SPMD program for a per-core edge capacity `cap`."""
    import concourse.bass as bass  # noqa: F401
    import concourse.bacc as bacc
    import concourse.mybir as mybir
    import concourse.tile as tile
    from concourse.tile import add_dep_helper

    f32 = mybir.dt.float32
    bf16 = mybir.dt.bfloat16
    i16 = mybir.dt.int16
    nblk = cap // 128

    nc = bacc.Bacc("TRN2", target_bir_lowering=False, debug=False,
                   num_devices=N_CORES, dynamic_dma_scratch_size=65536)

    ZT = nc.dram_tensor("zt", [D, N_DRUGS], bf16, kind="ExternalInput")
    ZB = nc.dram_tensor("zb", [D, BLK], bf16, kind="ExternalInput")
    WT = nc.dram_tensor("w", [D, D], bf16, kind="ExternalInput")
    DT = nc.dram_tensor("dvec", [128, KC], f32, kind="ExternalInput")
    MS = nc.dram_tensor("mask", [128, nblk, 128], bf16, kind="ExternalInput")
    IX = nc.dram_tensor("idx", [128, cap // 16], i16, kind="ExternalInput")
    OUT = nc.dram_tensor("out", [128, nblk], f32, kind="ExternalOutput")
    SD = nc.dram_tensor("s_scratch", [BLK, N_DRUGS], bf16)

    with tile.TileContext(nc) as tc:
        with (
            tc.tile_pool(name="big", bufs=1) as big,
            tc.tile_pool(name="sml", bufs=1) as sml,
            tc.tile_pool(name="stage", bufs=8) as stage,
            tc.tile_pool(name="psum", bufs=8, space="PSUM") as psum,
        ):
            d_sb = sml.tile([128, KC], f32)
            nc.sync.dma_start(d_sb[:], DT.ap())
            w_sb = sml.tile([128, KC, D], bf16)
            nc.sync.dma_start(w_sb[:], WT.ap().rearrange("(jc p) k -> p jc k", p=128))
            zb_sb = sml.tile([128, KC, BLK], bf16)
            nc.sync.dma_start(zb_sb[:], ZB.ap().rearrange("(kc p) m -> p kc m", p=128))
            zt_sb = big.tile([128, KC, N_DRUGS], bf16)
            nc.sync.dma_start(zt_sb[:], ZT.ap().rearrange("(kc p) n -> p kc n", p=128))
            # issue extraction-phase inputs now: they ride the SP HWDGE FIFO
            # ahead of the S stores and transfer during the matmul phase
            ix_sb = sml.tile([128, cap // 16], i16)
            nc.sync.dma_start(ix_sb[:], IX.ap())
            ms_sb = big.tile([128, nblk, 128], bf16)
            if ms_load:
                nc.sync.dma_start(ms_sb[:], MS.ap())
            else:
                nc.gpsimd.memset(ms_sb[:], 1.0)

            # U^T = z^T * d  (d is a per-partition scalar in each K chunk)
            for kc in range(KC):
                nc.vector.tensor_scalar_mul(zb_sb[:, kc, :], zb_sb[:, kc, :],
                                            d_sb[:, kc:kc + 1])
                nc.vector.tensor_scalar_mul(zt_sb[:, kc, :], zt_sb[:, kc, :],
                                            d_sb[:, kc:kc + 1])

            # A^T chunks for this core's block: a_sb[p, kc, m] = (U@W)[m, kc*128+p]
            a_sb = sml.tile([128, KC, BLK], bf16)
            for kc in range(KC):
                ps = psum.tile([128, BLK], f32, tag="ps")
                for jc in range(KC):
                    nc.tensor.matmul(ps[:], w_sb[:, jc, kc * 128:(kc + 1) * 128],
                                     zb_sb[:, jc, :],
                                     start=(jc == 0), stop=(jc == KC - 1))
                nc.scalar.copy(a_sb[:, kc, :], ps[:])

            # S block = A @ U^T, streamed to DRAM in [128, 512] tiles.
            # kc-outer over 8 PSUM banks: each lhsT slice streams 8 moving
            # tiles, cutting PE weight-reload overhead. Casts split ACT/DVE;
            # stores ride the ACT HWDGE ring, separate from the input loads.
            store_insts = []
            for mt in range(MT):
                pss = [psum.tile([128, 512], f32, tag="ps", name=f"ps_{mt}_{i}")
                       for i in range(NCH)]
                for kc in range(KC):
                    for nch in range(NCH):
                        nc.tensor.matmul(
                            pss[nch][:], a_sb[:, kc, mt * 128:(mt + 1) * 128],
                            zt_sb[:, kc, nch * 512:(nch + 1) * 512],
                            start=(kc == 0), stop=(kc == KC - 1))
                for nch in range(NCH):
                    s_sb = stage.tile([128, 512], bf16, tag="s_out")
                    if nch % 2 == 0:
                        nc.scalar.copy(s_sb[:], pss[nch][:])
                    else:
                        nc.vector.tensor_copy(s_sb[:], pss[nch][:])
                    st = nc.scalar.dma_start(
                        SD.ap()[mt * 128:(mt + 1) * 128, nch * 512:(nch + 1) * 512],
                        s_sb[:])
                    store_insts.append(st)

            # Per-edge extraction: gather 256B tokens (chunked so each
            # dma_gather fits the SWDGE descriptor ring), one-hot mask,
            # segmented reduce, sigmoid.
            g_sb = big.tile([128, nblk, 128], bf16)
            y_sb = sml.tile([128, nblk], f32)
            sd_view = SD.ap().rearrange("r (b c) -> (r b) c", c=128)
            p_sb = g_sb if inplace else big.tile([128, nblk, 128], bf16)
            CHUNK = 32  # blocks per dma_gather = 4096 indices
            for b0 in range(0, nblk, CHUNK):
                b1 = min(b0 + CHUNK, nblk)
                nidx = (b1 - b0) * 128
                if gather_mode == "real":
                    # single_packet=False: packed-single-packet mode faults the
                    # engine above 1024 idxs (64 descriptors/engine ceiling)
                    gi = nc.gpsimd.dma_gather(
                        g_sb[:, b0:b1, :], sd_view,
                        ix_sb[:, b0 * 8:b1 * 8],
                        num_idxs=nidx, num_idxs_reg=nidx, elem_size=128,
                        single_packet=False)
                    if dep_mode == "helper":
                        for st in store_insts:
                            add_dep_helper(gi.ins, st.ins,
                                           reason="gather reads S scratch")
                else:
                    nc.gpsimd.memset(g_sb[:, b0:b1, :], 0.5)
                if not tail:
                    continue
                nc.vector.tensor_tensor(p_sb[:, b0:b1, :], g_sb[:, b0:b1, :],
                                        ms_sb[:, b0:b1, :],
                                        op=mybir.AluOpType.mult)
                nc.vector.tensor_reduce(y_sb[:, b0:b1], p_sb[:, b0:b1, :],
                                        axis=mybir.AxisListType.X,
                                        op=mybir.AluOpType.add)
            o_sb = sml.tile([128, nblk], f32)
            if tail:
                nc.scalar.activation(o_sb[:], y_sb[:],
                                     mybir.ActivationFunctionType.Sigmoid)
            else:
                nc.vector.tensor_copy(o_sb[:], g_sb[:, :, 0])
            nc.sync.dma_start(OUT.ap(), o_sb[:])

    nc.compile()
    return nc


def _get_program(cap):
    if cap not in _cache:
        _cache[cap] = _build(cap)
    return _cache[cap]


def kernel(z_drug, global_weight, local_diag, batch_edges, edge_sub_type_idx,
           **_unused):
    from concourse.bass_utils import run_bass_kernel_spmd

    z = np.asarray(z_drug, np.float32)
    W = np.asarray(global_weight, np.float32)
    ld = np.asarray(local_diag, np.float32)
    e = np.asarray(batch_edges)
    sub = int(np.asarray(edge_sub_type_idx))
    d = ld[sub]
    assert z.shape == (N_DRUGS, D) and W.shape == (D, D)
    B = e.shape[1]
    e0 = e[0].astype(np.int64)
    e1 = e[1].astype(np.int64)

    zT = np.ascontiguousarray(z.T).astype(BF)          # [512, 4096]
    Wb = W.astype(BF)
    dT = np.ascontiguousarray(d.reshape(KC, 128).T)    # [128, 4] f32

    core = e0 // BLK
    counts = np.bincount(core, minlength=N_CORES)
    cap = max(128, int(-(-counts.max() // 128)) * 128)
    nblk = cap // 128

    in_maps = []
    positions = []
    one = BF(1.0)
    for c in range(N_CORES):
        sel = np.nonzero(core == c)[0]
        r = e0[sel] - c * BLK
        n = e1[sel]
        npad = cap - sel.size
        tok = np.zeros(cap, np.int16)
        tok[:sel.size] = (r * TPB + (n >> 7)).astype(np.int16)
        nm = np.zeros(cap, np.int64)
        nm[:sel.size] = n & 127
        # idx wrapped over 16 partitions, replicated to all 8 Q7 cores
        ixw = np.ascontiguousarray(
            np.tile(tok.reshape(cap // 16, 16).T, (8, 1)))
        mask = np.zeros((128, nblk, 128), BF)
        j = np.arange(cap)
        mask[j % 128, j // 128, nm] = one
        zB = np.ascontiguousarray(zT[:, c * BLK:(c + 1) * BLK])
        in_maps.append({"zt": zT, "zb": zB, "w": Wb, "dvec": dT,
                        "mask": mask, "idx": ixw})
        positions.append(sel)

    nc = _get_program(cap)
    res = run_bass_kernel_spmd(nc, in_maps, list(range(N_CORES)))

    out = np.empty(B, np.float32)
    for c in range(N_CORES):
        oc = np.asarray(res.results[c]["out"], np.float32)  # [128, nblk]
        flat = oc.T.reshape(-1)                             # j = b*128 + p
        out[positions[c]] = flat[:positions[c].size]
    return out


if __name__ == "__main__":
    dat = np.load("/root/problem/cached_io.npz")
    inputs = {k: dat[k] for k in ("z_drug", "global_weight", "local_diag",
                                  "batch_edges", "edge_sub_type_idx")}
    expected = dat["expected"]
    actual = kernel(**inputs)
    err = np.abs(actual - expected)
    print("max abs err:", err.max(), "mean:", err.mean())
    print("Relative error:", err.max() / np.abs(expected).max())



# revision 10
# speedup vs baseline: 1.6776x; 1.6776x over previous
"""Trainium2 Bass kernel for the Dedicom decoder problem.

Math: with U = z * d (row-wise scale by the selected local_diag row),
    score_b = ((z[e0]*d) @ W) * d . z[e1] = U[e0] @ W @ U[e1]^T
so all-pairs scores S = (U @ W) @ U^T contain every edge score.  Core c
computes the 512-row block S[512c:512c+512, :] entirely on-chip:
A^T = W^T-contracted block (PE), then S in 8 waves of [128 rows, 2048
cols] (PE -> PSUM -> SBUF bf16 casts split across DVE/ACT).  Per-edge
extraction runs on the otherwise-idle GPSIMD engine via indirect_copy
(free-dim gather; each 16-partition group shares an index list), and the
16-way partition redundancy is resolved by a host-built one-hot mask +
a PE segment-sum (lhsT = 16-partition segment indicator), followed by
sigmoid on ACT.  No DRAM scratch, no DMA gather, no big masks.
Edges are bucketed on the host by (core, wave, group); results are
unscattered on the host.
"""

import numpy as np
import ml_dtypes

BF = ml_dtypes.bfloat16

N_DRUGS = 4096
D = 512
N_CORES = 8
BLK = N_DRUGS // N_CORES  # 512 rows of S per core
KC = D // 128             # 4 contraction chunks
MT = BLK // 128           # 4 row tiles of the core's S block
NW = 8                    # S waves: (col-half, row-tile)
WCOL = N_DRUGS // 2       # 2048 columns per wave

_cache = {}


def _build(nv):
    """Build + compile the SPMD program; `nv` = slots per (wave, group)."""
    import concourse.bass as bass  # noqa: F401
    import concourse.bacc as bacc
    import concourse.mybir as mybir
    import concourse.tile as tile

    f32 = mybir.dt.float32
    bf16 = mybir.dt.bfloat16
    u16 = mybir.dt.uint16

    nc = bacc.Bacc("TRN2", target_bir_lowering=False, debug=False,
                   num_devices=N_CORES)

    ZT = nc.dram_tensor("zt", [D, N_DRUGS], bf16, kind="ExternalInput")
    ZB = nc.dram_tensor("zb", [D, BLK], bf16, kind="ExternalInput")
    WT = nc.dram_tensor("w", [D, D], bf16, kind="ExternalInput")
    IX = nc.dram_tensor("idx", [128, NW, nv // 16], u16, kind="ExternalInput")
    MS = nc.dram_tensor("mask", [128, NW, nv], bf16, kind="ExternalInput")
    SG = nc.dram_tensor("seg", [128, 8], bf16, kind="ExternalInput")
    OUT = nc.dram_tensor("out", [8, NW * nv], f32, kind="ExternalOutput")

    with tile.TileContext(nc) as tc:
        with (
            tc.tile_pool(name="big", bufs=1) as big,
            tc.tile_pool(name="sml", bufs=1) as sml,
            tc.tile_pool(name="psum", bufs=8, space="PSUM") as psum,
        ):
            zb_sb = sml.tile([128, KC, BLK], bf16)
            nc.sync.dma_start(zb_sb[:], ZB.ap().rearrange("(kc p) m -> p kc m", p=128))
            w_sb = sml.tile([128, KC, D], bf16)
            nc.sync.dma_start(w_sb[:], WT.ap().rearrange("(jc p) k -> p jc k", p=128))
            ix_w = []
            for w in range(NW):
                ixt = sml.tile([128, nv // 16], u16, name=f"ix_{w}")
                nc.scalar.dma_start(ixt[:], IX.ap()[:, w, :])
                ix_w.append(ixt)
            sg_sb = sml.tile([128, 8], bf16)
            nc.scalar.dma_start(sg_sb[:], SG.ap())
            ms_sb = big.tile([128, NW, nv], bf16)
            nc.scalar.dma_start(ms_sb[:], MS.ap())
            # zt in two halves so wave 0 can start after the low half lands
            zt_sb = big.tile([128, KC, N_DRUGS], bf16)
            zt_v = ZT.ap().rearrange("(kc p) n -> p kc n", p=128)
            nc.sync.dma_start(zt_sb[:, :, 0:WCOL], zt_v[:, :, 0:WCOL])
            nc.sync.dma_start(zt_sb[:, :, WCOL:], zt_v[:, :, WCOL:])

            # A^T chunks: a_sb[p, kc, m] = (U_blk @ W)[m, kc*128+p]
            a_sb = sml.tile([128, KC, BLK], bf16)
            for kc in range(KC):
                ps = psum.tile([128, BLK], f32, tag="ps", bufs=6)
                for jc in range(KC):
                    nc.tensor.matmul(ps[:], w_sb[:, jc, kc * 128:(kc + 1) * 128],
                                     zb_sb[:, jc, :],
                                     start=(jc == 0), stop=(jc == KC - 1))
                if kc % 2 == 0:
                    nc.scalar.copy(a_sb[:, kc, :], ps[:])
                else:
                    nc.vector.tensor_copy(a_sb[:, kc, :], ps[:])

            # S waves: wave w = (half = w>>2, mt = w&3), cols
            # [half*2048, half*2048+2048).  PE -> PSUM -> SBUF bf16 ->
            # gpsimd indirect_copy -> masked one-hot -> later seg-reduce.
            p_sb = big.tile([128, NW, nv], bf16)
            for w in range(NW):
                half, mt = w >> 2, w & 3
                c0 = half * WCOL
                sw = big.tile([128, WCOL], bf16, name=f"sw_{w}", tag="sw",
                              bufs=3)
                for nch in range(4):
                    ps = psum.tile([128, 512], f32, tag="ps", bufs=6,
                                   name=f"s_{w}_{nch}")
                    for jc in range(KC):
                        nc.tensor.matmul(
                            ps[:], a_sb[:, jc, mt * 128:(mt + 1) * 128],
                            zt_sb[:, jc, c0 + nch * 512:c0 + (nch + 1) * 512],
                            start=(jc == 0), stop=(jc == KC - 1))
                    if nch % 2 == 0:
                        nc.scalar.copy(sw[:, nch * 512:(nch + 1) * 512], ps[:])
                    else:
                        nc.vector.tensor_copy(sw[:, nch * 512:(nch + 1) * 512],
                                              ps[:])
                g_w = big.tile([128, nv], bf16, name=f"g_{w}")
                nc.gpsimd.indirect_copy(g_w[:], sw[:], ix_w[w][:],
                                        i_know_ap_gather_is_preferred=True)
                nc.vector.tensor_tensor(p_sb[:, w, :], g_w[:],
                                        ms_sb[:, w, :],
                                        op=mybir.AluOpType.mult)

            # resolve the 16-way redundancy: out[g, col] = sum over the 16
            # partitions of group g of p_sb, then sigmoid.
            o_sb = sml.tile([8, NW * nv], f32)
            pf = p_sb[:].rearrange("p w v -> p (w v)")
            CH = 512
            tot = NW * nv
            for c0 in range(0, tot, CH):
                c1 = min(c0 + CH, tot)
                pr = psum.tile([8, CH], f32, tag="seg", bufs=2)
                nc.tensor.matmul(pr[:, :c1 - c0], sg_sb[:], pf[:, c0:c1],
                                 start=True, stop=True)
                nc.scalar.activation(o_sb[:, c0:c1], pr[:, :c1 - c0],
                                     mybir.ActivationFunctionType.Sigmoid)
            nc.sync.dma_start(OUT.ap(), o_sb[:])

    nc.compile()
    return nc


def _get_program(nv):
    if nv not in _cache:
        _cache[nv] = _build(nv)
    return _cache[nv]


def kernel(z_drug, global_weight, local_diag, batch_edges, edge_sub_type_idx,
           **_unused):
    from concourse.bass_utils import run_bass_kernel_spmd

    z = np.asarray(z_drug, np.float32)
    W = np.asarray(global_weight, np.float32)
    ld = np.asarray(local_diag, np.float32)
    e = np.asarray(batch_edges)
    sub = int(np.asarray(edge_sub_type_idx))
    d = ld[sub]
    assert z.shape == (N_DRUGS, D) and W.shape == (D, D)
    B = e.shape[1]
    e0 = e[0].astype(np.int64)
    e1 = e[1].astype(np.int64)

    U = z * d                                           # [4096, 512] f32
    zT = np.ascontiguousarray(U.T).astype(BF)           # [512, 4096]
    Wb = W.astype(BF)

    core = e0 // BLK
    r = e0 - core * BLK
    n = e1
    w = (n >> 11) * 4 + (r >> 7)                        # wave
    g = (r & 127) >> 4                                  # 16-partition group
    lo = r & 15
    idx = n & 2047

    # slot i within each (core, wave, group) bucket
    order = np.lexsort((np.arange(B), g, w, core))
    cs, ws, gs = core[order], w[order], g[order]
    key = (cs * NW + ws) * 8 + gs
    start = np.searchsorted(key, np.arange(N_CORES * NW * 8), side="left")
    end = np.searchsorted(key, np.arange(N_CORES * NW * 8), side="right")
    counts = end - start
    slot = np.arange(B) - start[key]
    nv = max(16, int(-(-counts.max() // 16)) * 16)

    seg = np.zeros((128, 8), BF)
    for gg in range(8):
        seg[16 * gg:16 * gg + 16, gg] = BF(1.0)

    in_maps = []
    positions = []
    for c in range(N_CORES):
        m = order[cs == c]
        wc, gc, ic = w[m], g[m], slot[cs == c]
        ix = np.zeros((128, NW, nv // 16), np.uint16)
        ix[16 * gc + ic % 16, wc, ic // 16] = idx[m].astype(np.uint16)
        mask = np.zeros((128, NW, nv), BF)
        mask[16 * gc + lo[m], wc, ic] = BF(1.0)
        zB = np.ascontiguousarray(zT[:, c * BLK:(c + 1) * BLK])
        in_maps.append({"zt": zT, "zb": zB, "w": Wb, "idx": ix,
                        "mask": mask, "seg": seg})
        positions.append((m, gc, wc * nv + ic))

    nc = _get_program(nv)
    res = run_bass_kernel_spmd(nc, in_maps, list(range(N_CORES)))

    out = np.empty(B, np.float32)
    for c in range(N_CORES):
        oc = np.asarray(res.results[c]["out"], np.float32)  # [8, NW*nv]
        m, gc, col = positions[c]
        out[m] = oc[gc, col]
    return out


if __name__ == "__main__":
    dat = np.load("/root/problem/cached_io.npz")
    inputs = {k: dat[k] for k in ("z_drug", "global_weight", "local_diag",
                                  "batch_edges", "edge_sub_type_idx")}
    expected = dat["expected"]
    actual = kernel(**inputs)
    err = np.abs(actual - expected)
    print("max abs err:", err.max(), "mean:", err.mean())
    print("Relative error:", err.max() / np.abs(expected).max())


# revision 15
# speedup vs baseline: 2.4302x; 1.4486x over previous
"""Trainium2 Bass kernel for the Dedicom decoder problem.

Math: with U = z * d (row-wise scale by the selected local_diag row),
    score_b = ((z[e0]*d) @ W) * d . z[e1] = U[e0] @ W @ U[e1]^T
so all-pairs scores S = (U @ W) @ U^T contain every edge score.  Core c
computes the 512-row block S[512c:512c+512, :] entirely on-chip in fp8
(DoubleRow matmuls, inputs pre-scaled x16 on the host; scores only span
|S| < 0.5 so fp8 keeps sigmoid error ~1e-3): A = U_blk @ W (PE), then S
in 8 waves of [128 rows, 2048 cols] (PE -> PSUM -> SBUF bf16 casts split
across DVE/ACT).  Per-edge extraction runs on the otherwise-idle GPSIMD
engine via indirect_copy (free-dim gather; each 16-partition group
shares an index list); the 16-way partition redundancy is resolved by a
host one-hot mask (DVE mult) + a PE segment-sum (lhsT = 16-partition
segment indicator), then sigmoid(x/4096) on ACT.  No DRAM scratch, no
DMA gather, no big masks.  Edges are bucketed on the host by
(core, wave, group); results are unscattered on the host.
"""

import numpy as np
import ml_dtypes

BF = ml_dtypes.bfloat16
F8 = ml_dtypes.float8_e4m3fn

N_DRUGS = 4096
D = 512
N_CORES = 8
BLK = N_DRUGS // N_CORES  # 512 rows of S per core
KC = D // 128             # 4 contraction chunks
MT = BLK // 128           # 4 row tiles of the core's S block
NW = 8                    # S waves: (col-half, row-tile)
WCOL = N_DRUGS // 2       # 2048 columns per wave
SU = 16.0                 # host pre-scale on U and W (fp8 dynamic range)

_cache = {}


def _build(nv):
    """Build + compile the SPMD program; `nv` = slots per (wave, group)."""
    import concourse.bass as bass  # noqa: F401
    import concourse.bacc as bacc
    import concourse.mybir as mybir
    import concourse.tile as tile

    f32 = mybir.dt.float32
    bf16 = mybir.dt.bfloat16
    fp8 = mybir.dt.float8e4
    u16 = mybir.dt.uint16
    DR = mybir.MatmulPerfMode.DoubleRow

    nc = bacc.Bacc("TRN2", target_bir_lowering=False, debug=False,
                   num_devices=N_CORES)

    ZT = nc.dram_tensor("zt", [D, N_DRUGS], fp8, kind="ExternalInput")
    ZB = nc.dram_tensor("zb", [D, BLK], fp8, kind="ExternalInput")
    WT = nc.dram_tensor("w", [D, D], fp8, kind="ExternalInput")
    IX = nc.dram_tensor("idx", [128, NW, nv // 16], u16, kind="ExternalInput")
    MS = nc.dram_tensor("mask", [128, NW, nv], bf16, kind="ExternalInput")
    SG = nc.dram_tensor("seg", [128, 8], bf16, kind="ExternalInput")
    OUT = nc.dram_tensor("out", [8, NW * nv], f32, kind="ExternalOutput")

    with tile.TileContext(nc) as tc:
        with (
            tc.tile_pool(name="big", bufs=1) as big,
            tc.tile_pool(name="sml", bufs=1) as sml,
            tc.tile_pool(name="psum", bufs=8, space="PSUM") as psum,
        ):
            zb_sb = sml.tile([128, KC, BLK], fp8)
            nc.sync.dma_start(zb_sb[:], ZB.ap().rearrange("(kc p) m -> p kc m", p=128))
            w_sb = sml.tile([128, KC, D], fp8)
            nc.sync.dma_start(w_sb[:], WT.ap().rearrange("(jc p) k -> p jc k", p=128))
            # zt in two halves so wave 0 can start after the low half lands
            zt_sb = big.tile([128, KC, N_DRUGS], fp8)
            zt_v = ZT.ap().rearrange("(kc p) n -> p kc n", p=128)
            nc.sync.dma_start(zt_sb[:, :, 0:WCOL], zt_v[:, :, 0:WCOL])
            ix_w = []
            for w in range(NW):
                ixt = sml.tile([128, nv // 16], u16, name=f"ix_{w}")
                nc.scalar.dma_start(ixt[:], IX.ap()[:, w, :])
                ix_w.append(ixt)
            sg_sb = sml.tile([128, 8], bf16)
            nc.scalar.dma_start(sg_sb[:], SG.ap())
            ms_sb = big.tile([128, NW, nv], bf16)
            nc.scalar.dma_start(ms_sb[:], MS.ap())
            nc.sync.dma_start(zt_sb[:, :, WCOL:], zt_v[:, :, WCOL:])

            # A^T chunks: a8[p, kc, m] = (U_blk*SU @ W*SU)[m, kc*128+p]
            a8_sb = sml.tile([128, KC, BLK], fp8)
            for kc in range(KC):
                ps = psum.tile([128, BLK], f32, tag="ps", bufs=6)
                for jc2 in range(2):
                    nc.tensor.matmul(
                        ps[:],
                        w_sb[:, 2 * jc2:2 * jc2 + 2, kc * 128:(kc + 1) * 128],
                        zb_sb[:, 2 * jc2:2 * jc2 + 2, :],
                        start=(jc2 == 0), stop=(jc2 == 1), perf_mode=DR)
                if kc % 2 == 0:
                    nc.scalar.copy(a8_sb[:, kc, :], ps[:])
                else:
                    nc.vector.tensor_copy(a8_sb[:, kc, :], ps[:])

            # S waves: wave w = (half = w>>2, mt = w&3), cols
            # [half*2048, half*2048+2048).  PE -> PSUM -> SBUF bf16 ->
            # gpsimd indirect_copy -> one-hot mult -> seg-sum -> sigmoid.
            o_sb = sml.tile([8, NW * nv], f32)
            p_sb = big.tile([128, NW, nv], bf16)
            for w in range(NW):
                half, mt = w >> 2, w & 3
                c0 = half * WCOL
                sw = big.tile([128, WCOL], bf16, name=f"sw_{w}", tag="sw",
                              bufs=3)
                for nch in range(4):
                    ps = psum.tile([128, 512], f32, tag="ps", bufs=6,
                                   name=f"s_{w}_{nch}")
                    for jc2 in range(2):
                        nc.tensor.matmul(
                            ps[:],
                            a8_sb[:, 2 * jc2:2 * jc2 + 2,
                                  mt * 128:(mt + 1) * 128],
                            zt_sb[:, 2 * jc2:2 * jc2 + 2,
                                  c0 + nch * 512:c0 + (nch + 1) * 512],
                            start=(jc2 == 0), stop=(jc2 == 1), perf_mode=DR)
                    if nch % 2 == 0:
                        nc.scalar.copy(sw[:, nch * 512:(nch + 1) * 512], ps[:])
                    else:
                        nc.vector.tensor_copy(sw[:, nch * 512:(nch + 1) * 512],
                                              ps[:])
                g_w = big.tile([128, nv], bf16, name=f"g_{w}")
                nc.gpsimd.indirect_copy(g_w[:], sw[:], ix_w[w][:],
                                        i_know_ap_gather_is_preferred=True)
                nc.vector.tensor_tensor(p_sb[:, w, :], g_w[:],
                                        ms_sb[:, w, :],
                                        op=mybir.AluOpType.mult)
                pr = psum.tile([8, nv], f32, tag="seg", name=f"pr_{w}",
                               bufs=2)
                nc.tensor.matmul(pr[:], sg_sb[:], p_sb[:, w, :],
                                 start=True, stop=True)
                nc.scalar.activation(o_sb[:, w * nv:(w + 1) * nv], pr[:],
                                     mybir.ActivationFunctionType.Sigmoid,
                                     scale=1.0 / (SU * SU * SU))
            nc.sync.dma_start(OUT.ap(), o_sb[:])

    nc.compile()
    return nc


def _get_program(nv):
    if nv not in _cache:
        _cache[nv] = _build(nv)
    return _cache[nv]


def kernel(z_drug, global_weight, local_diag, batch_edges, edge_sub_type_idx,
           **_unused):
    from concourse.bass_utils import run_bass_kernel_spmd

    z = np.asarray(z_drug, np.float32)
    W = np.asarray(global_weight, np.float32)
    ld = np.asarray(local_diag, np.float32)
    e = np.asarray(batch_edges)
    sub = int(np.asarray(edge_sub_type_idx))
    d = ld[sub]
    assert z.shape == (N_DRUGS, D) and W.shape == (D, D)
    B = e.shape[1]
    e0 = e[0].astype(np.int64)
    e1 = e[1].astype(np.int64)

    U = z * d * SU                                      # [4096, 512] f32
    zT = np.ascontiguousarray(U.T).astype(F8)           # [512, 4096] fp8
    Wb = (W * SU).astype(F8)

    core = e0 // BLK
    r = e0 - core * BLK
    n = e1
    w = (n >> 11) * 4 + (r >> 7)                        # wave
    g = (r & 127) >> 4                                  # 16-partition group
    lo = r & 15
    idx = n & 2047

    # slot i within each (core, wave, group) bucket
    order = np.lexsort((np.arange(B), g, w, core))
    cs, ws, gs = core[order], w[order], g[order]
    key = (cs * NW + ws) * 8 + gs
    start = np.searchsorted(key, np.arange(N_CORES * NW * 8), side="left")
    counts = np.bincount(key, minlength=N_CORES * NW * 8)
    slot = np.arange(B) - start[key]
    nv = max(16, int(-(-counts.max() // 16)) * 16)

    seg = np.zeros((128, 8), BF)
    for gg in range(8):
        seg[16 * gg:16 * gg + 16, gg] = BF(1.0)

    in_maps = []
    positions = []
    for c in range(N_CORES):
        m = order[cs == c]
        wc, gc, ic = w[m], g[m], slot[cs == c]
        ix = np.zeros((128, NW, nv // 16), np.uint16)
        ix[16 * gc + ic % 16, wc, ic // 16] = idx[m].astype(np.uint16)
        mask = np.zeros((128, NW, nv), BF)
        mask[16 * gc + lo[m], wc, ic] = BF(1.0)
        zB = np.ascontiguousarray(zT[:, c * BLK:(c + 1) * BLK])
        in_maps.append({"zt": zT, "zb": zB, "w": Wb, "idx": ix,
                        "mask": mask, "seg": seg})
        positions.append((m, gc, wc * nv + ic))

    nc = _get_program(nv)
    res = run_bass_kernel_spmd(nc, in_maps, list(range(N_CORES)))

    out = np.empty(B, np.float32)
    for c in range(N_CORES):
        oc = np.asarray(res.results[c]["out"], np.float32)  # [8, NW*nv]
        m, gc, col = positions[c]
        out[m] = oc[gc, col]
    return out


if __name__ == "__main__":
    dat = np.load("/root/problem/cached_io.npz")
    inputs = {k: dat[k] for k in ("z_drug", "global_weight", "local_diag",
                                  "batch_edges", "edge_sub_type_idx")}
    expected = dat["expected"]
    actual = kernel(**inputs)
    err = np.abs(actual - expected)
    print("max abs err:", err.max(), "mean:", err.mean())
    print("Relative error:", err.max() / np.abs(expected).max())


# revision 17
# speedup vs baseline: 2.6384x; 1.0856x over previous
"""Trainium2 Bass kernel for the Dedicom decoder problem.

Math: with U = z * d (row-wise scale by the selected local_diag row),
    score_b = ((z[e0]*d) @ W) * d . z[e1] = U[e0] @ W @ U[e1]^T
so all-pairs scores S = A @ U^T with A = U @ W contain every edge score.
A is edge-independent, so the host precomputes it (f32) and ships
A^T x256 and U^T x16 in fp8 (scores only span |S| < 0.5, so fp8 keeps
the sigmoid error ~1e-3).  Core c computes its 512-row block of S as 32
DoubleRow matmuls in 16 column-units of [128 rows, 1024 cols]
(PE -> PSUM -> SBUF bf16 casts split across DVE/ACT).  Per-edge
extraction runs on the otherwise-idle GPSIMD engine via indirect_copy
(free-dim gather; each 16-partition group shares an index list); the
16-way partition redundancy is resolved by a host one-hot mask (DVE
mult) + a PE segment-sum (lhsT = 16-partition segment indicator), then
sigmoid(x/4096) on ACT.  All inputs arrive in 4 packed DMAs (HWDGE
issue cost dominates small transfers); index lists are unpacked
on-chip.  No DRAM scratch, no DMA gather, no big masks.  Edges are
bucketed on the host by (core, unit, group); results are unscattered
on the host.
"""

import numpy as np
import ml_dtypes

BF = ml_dtypes.bfloat16
F8 = ml_dtypes.float8_e4m3fn

N_DRUGS = 4096
D = 512
N_CORES = 8
BLK = N_DRUGS // N_CORES  # 512 rows of S per core
KC = D // 128             # 4 contraction chunks
MT = BLK // 128           # 4 row tiles of the core's S block
NU = 16                   # extraction units: (col-quarter, row-tile)
UCOL = 1024               # columns per unit
SU = 16.0                 # host pre-scale on U and W (fp8 dynamic range)
PKW = BLK + N_DRUGS       # packed matrix cols: a8T | zt

_cache = {}


def _build(nv):
    """Build + compile the SPMD program; `nv` = slots per (unit, group)."""
    import concourse.bass as bass  # noqa: F401
    import concourse.bacc as bacc
    import concourse.mybir as mybir
    import concourse.tile as tile

    f32 = mybir.dt.float32
    bf16 = mybir.dt.bfloat16
    fp8 = mybir.dt.float8e4
    u16 = mybir.dt.uint16
    DR = mybir.MatmulPerfMode.DoubleRow

    nvi = nv // 16
    # aux pack (bf16 cols): mask [NU*nv] | seg [8] | idx-as-bf16 [NU*nvi]
    AXW = NU * nv + 8 + NU * nvi

    nc = bacc.Bacc("TRN2", target_bir_lowering=False, debug=False,
                   num_devices=N_CORES)

    PK = nc.dram_tensor("pack", [D, PKW], fp8, kind="ExternalInput")
    AX = nc.dram_tensor("aux", [128, AXW], bf16, kind="ExternalInput")
    OUT = nc.dram_tensor("out", [8, NU * nv], f32, kind="ExternalOutput")

    with tile.TileContext(nc) as tc:
        with (
            tc.tile_pool(name="big", bufs=1) as big,
            tc.tile_pool(name="sml", bufs=1) as sml,
            tc.tile_pool(name="psum", bufs=8, space="PSUM") as psum,
        ):
            pk_sb = big.tile([128, KC, PKW], fp8)
            pk_v = PK.ap().rearrange("(kc p) n -> p kc n", p=128)
            C1 = BLK + UCOL        # dma0: a8T + zt quarter 0
            C2 = BLK + 3 * UCOL    # dma1: zt quarters 1-2
            nc.sync.dma_start(pk_sb[:, :, 0:C1], pk_v[:, :, 0:C1])
            ax_sb = big.tile([128, AXW], bf16)
            nc.scalar.dma_start(ax_sb[:], AX.ap())
            nc.sync.dma_start(pk_sb[:, :, C1:C2], pk_v[:, :, C1:C2])
            nc.sync.dma_start(pk_sb[:, :, C2:], pk_v[:, :, C2:])

            a8 = pk_sb[:, :, 0:BLK]
            zt = pk_sb[:, :, BLK:]
            ms_v = ax_sb[:, 0:NU * nv].rearrange("p (u v) -> p u v", u=NU)
            sg_sb = sml.tile([128, 8], bf16)
            nc.vector.tensor_copy(sg_sb[:], ax_sb[:, NU * nv:NU * nv + 8])
            ix_w = []
            x0 = NU * nv + 8
            for u in range(NU):
                ixt = sml.tile([128, nvi], u16, name=f"ix_{u}")
                nc.vector.tensor_copy(
                    ixt[:],
                    ax_sb[:, x0 + u * nvi:x0 + (u + 1) * nvi].bitcast(u16))
                ix_w.append(ixt)

            # S units: unit u = (q = u>>2, mt = u&3), cols
            # [q*1024, q*1024+1024).  PE -> PSUM -> SBUF bf16 ->
            # gpsimd indirect_copy -> one-hot mult; seg-sum + sigmoid
            # per unit-pair.
            o_sb = sml.tile([8, NU * nv], f32)
            p_sb = big.tile([128, NU, nv], bf16)
            for u in range(NU):
                q, mt = u >> 2, u & 3
                c0 = q * UCOL
                sw = big.tile([128, UCOL], bf16, name=f"sw_{u}", tag="sw",
                              bufs=4)
                for nch in range(2):
                    ps = psum.tile([128, 512], f32, tag="ps", bufs=6,
                                   name=f"s_{u}_{nch}")
                    for jc2 in range(2):
                        nc.tensor.matmul(
                            ps[:],
                            a8[:, 2 * jc2:2 * jc2 + 2,
                               mt * 128:(mt + 1) * 128],
                            zt[:, 2 * jc2:2 * jc2 + 2,
                               c0 + nch * 512:c0 + (nch + 1) * 512],
                            start=(jc2 == 0), stop=(jc2 == 1), perf_mode=DR)
                    if (2 * u + nch) % 3 == 0:
                        nc.scalar.copy(sw[:, nch * 512:(nch + 1) * 512], ps[:])
                    else:
                        nc.vector.tensor_copy(sw[:, nch * 512:(nch + 1) * 512],
                                              ps[:])
                g_w = big.tile([128, nv], bf16, name=f"g_{u}")
                nc.gpsimd.indirect_copy(g_w[:], sw[:], ix_w[u][:],
                                        i_know_ap_gather_is_preferred=True)
                nc.vector.tensor_tensor(p_sb[:, u, :], g_w[:],
                                        ms_v[:, u, :],
                                        op=mybir.AluOpType.mult)
                if u % 2 == 1:
                    pr = psum.tile([8, 2 * nv], f32, tag="seg",
                                   name=f"pr_{u}", bufs=2)
                    pru = p_sb[:, u - 1:u + 1, :].rearrange(
                        "p u v -> p (u v)")
                    nc.tensor.matmul(pr[:], sg_sb[:], pru,
                                     start=True, stop=True)
                    nc.scalar.activation(
                        o_sb[:, (u - 1) * nv:(u + 1) * nv], pr[:],
                        mybir.ActivationFunctionType.Sigmoid,
                        scale=1.0 / (SU * SU * SU))
            nc.sync.dma_start(OUT.ap(), o_sb[:])

    nc.compile()
    return nc


def _get_program(nv):
    if nv not in _cache:
        _cache[nv] = _build(nv)
    return _cache[nv]


def kernel(z_drug, global_weight, local_diag, batch_edges, edge_sub_type_idx,
           **_unused):
    from concourse.bass_utils import run_bass_kernel_spmd

    z = np.asarray(z_drug, np.float32)
    W = np.asarray(global_weight, np.float32)
    ld = np.asarray(local_diag, np.float32)
    e = np.asarray(batch_edges)
    sub = int(np.asarray(edge_sub_type_idx))
    d = ld[sub]
    assert z.shape == (N_DRUGS, D) and W.shape == (D, D)
    B = e.shape[1]
    e0 = e[0].astype(np.int64)
    e1 = e[1].astype(np.int64)

    U = z * d                                           # [4096, 512] f32
    A = U @ W                                           # [4096, 512] f32
    zT8 = np.ascontiguousarray((U.T * SU)).astype(F8)   # [512, 4096] fp8

    core = e0 // BLK
    r = e0 - core * BLK
    n = e1
    w = (n >> 10) * 4 + (r >> 7)                        # extraction unit
    g = (r & 127) >> 4                                  # 16-partition group
    lo = r & 15
    idx = n & 1023

    # slot i within each (core, unit, group) bucket
    order = np.lexsort((np.arange(B), g, w, core))
    cs, ws, gs = core[order], w[order], g[order]
    key = (cs * NU + ws) * 8 + gs
    start = np.searchsorted(key, np.arange(N_CORES * NU * 8), side="left")
    counts = np.bincount(key, minlength=N_CORES * NU * 8)
    slot = np.arange(B) - start[key]
    nv = max(16, int(-(-counts.max() // 16)) * 16)
    nvi = nv // 16

    seg = np.zeros((128, 8), BF)
    for gg in range(8):
        seg[16 * gg:16 * gg + 16, gg] = BF(1.0)

    in_maps = []
    positions = []
    for c in range(N_CORES):
        m = order[cs == c]
        wc, gc, ic = w[m], g[m], slot[cs == c]
        ix = np.zeros((128, NU, nvi), np.uint16)
        ix[16 * gc + ic % 16, wc, ic // 16] = idx[m].astype(np.uint16)
        mask = np.zeros((128, NU, nv), BF)
        mask[16 * gc + lo[m], wc, ic] = BF(1.0)
        a8T = np.ascontiguousarray(
            (A[c * BLK:(c + 1) * BLK].T * (SU * SU))).astype(F8)
        pack = np.concatenate([a8T, zT8], axis=1)       # [512, PKW] fp8
        aux = np.concatenate(
            [mask.reshape(128, -1),
             seg,
             ix.reshape(128, -1).view(BF)], axis=1)     # [128, AXW] bf16
        in_maps.append({"pack": pack, "aux": aux})
        positions.append((m, gc, wc * nv + ic))

    nc = _get_program(nv)
    res = run_bass_kernel_spmd(nc, in_maps, list(range(N_CORES)))

    out = np.empty(B, np.float32)
    for c in range(N_CORES):
        oc = np.asarray(res.results[c]["out"], np.float32)  # [8, NU*nv]
        m, gc, col = positions[c]
        out[m] = oc[gc, col]
    return out


if __name__ == "__main__":
    dat = np.load("/root/problem/cached_io.npz")
    inputs = {k: dat[k] for k in ("z_drug", "global_weight", "local_diag",
                                  "batch_edges", "edge_sub_type_idx")}
    expected = dat["expected"]
    actual = kernel(**inputs)
    err = np.abs(actual - expected)
    print("max abs err:", err.max(), "mean:", err.mean())
    print("Relative error:", err.max() / np.abs(expected).max())


# revision 21
# speedup vs baseline: 3.3659x; 1.2758x over previous
"""Trainium2 Bass kernel for the Dedicom decoder problem.

Math: with U = z * d (row-wise scale by the selected local_diag row),
    score_b = ((z[e0]*d) @ W) * d . z[e1] = U[e0] @ W @ U[e1]^T
so all-pairs scores S = A @ U^T with A = U @ W contain every edge score.
A is edge-independent, so the host precomputes it (f32) and ships
A^T x256 and U^T x16 in fp8 (scores only span |S| < 0.5, so fp8 keeps
the sigmoid error ~1e-3).  Core c computes its 512-row block of S as 64
DoubleRow matmuls in 16 column-units of [128 rows, 1024 cols], drained
PSUM -> SBUF bf16 by one cast per unit (alternating DVE/ACT).
Per-edge extraction runs on the otherwise-idle GPSIMD engine via
indirect_copy over the unit's data BITCAST TO F32 (so the scan is 512
pair-elements, halving GPSIMD time); each fetched f32 is a pair of
scores and the index lists are parity-sorted so a strided one-hot
multiply (DVE) picks the right half.  The 16-way partition redundancy
of indirect_copy is resolved by that same host mask + a PE segment-sum
(lhsT = 16-partition segment indicator), then sigmoid(x/4096) on ACT.
All inputs arrive in 4 packed DMAs (HWDGE issue cost dominates small
transfers); index lists are unpacked on-chip; dummy matmuls warm the PE
p-state during the first DMA.  Edges are bucketed on the host by
(core, unit, group, column-parity); results are unscattered on the
host.
"""

import numpy as np
import ml_dtypes

BF = ml_dtypes.bfloat16
F8 = ml_dtypes.float8_e4m3fn

N_DRUGS = 4096
D = 512
N_CORES = 8
BLK = N_DRUGS // N_CORES  # 512 rows of S per core
KC = D // 128             # 4 contraction chunks
MT = BLK // 128           # 4 row tiles of the core's S block
NU = 16                   # extraction units: (col-quarter, row-tile)
UCOL = 1024               # columns per unit
SU = 16.0                 # host pre-scale on U and W (fp8 dynamic range)
PKW = BLK + N_DRUGS       # packed matrix cols: a8T | zt

_cache = {}


def _build(nv):
    """Build + compile the SPMD program; `nv` = 2*nv2 slots per
    (unit, group): even-parity slots then odd-parity slots."""
    import concourse.bass as bass  # noqa: F401
    import concourse.bacc as bacc
    import concourse.mybir as mybir
    import concourse.tile as tile

    f32 = mybir.dt.float32
    bf16 = mybir.dt.bfloat16
    fp8 = mybir.dt.float8e4
    u16 = mybir.dt.uint16
    DR = mybir.MatmulPerfMode.DoubleRow

    nv2 = nv // 2
    nvi = nv // 16
    # aux pack (bf16 cols): mask [NU*nv] | seg [8] | idx-as-bf16 [NU*nvi]
    AXW = NU * nv + 8 + NU * nvi

    nc = bacc.Bacc("TRN2", target_bir_lowering=False, debug=False,
                   num_devices=N_CORES)

    PK = nc.dram_tensor("pack", [D, PKW], fp8, kind="ExternalInput")
    AX = nc.dram_tensor("aux", [128, AXW], bf16, kind="ExternalInput")
    OUT = nc.dram_tensor("out", [8, NU * nv], f32, kind="ExternalOutput")

    with tile.TileContext(nc) as tc:
        with (
            tc.tile_pool(name="big", bufs=1) as big,
            tc.tile_pool(name="sml", bufs=1) as sml,
            tc.tile_pool(name="psum", bufs=8, space="PSUM") as psum,
        ):
            pk_sb = big.tile([128, KC, PKW], fp8)
            pk_v = PK.ap().rearrange("(kc p) n -> p kc n", p=128)
            C1 = BLK + UCOL        # dma0: a8T + zt quarter 0
            C2 = BLK + 3 * UCOL    # dma1: zt quarters 1-2
            nc.sync.dma_start(pk_sb[:, :, 0:C1], pk_v[:, :, 0:C1])
            ax_sb = big.tile([128, AXW], bf16)
            nc.scalar.dma_start(ax_sb[:], AX.ap())
            nc.sync.dma_start(pk_sb[:, :, C1:C2], pk_v[:, :, C1:C2])
            nc.sync.dma_start(pk_sb[:, :, C2:], pk_v[:, :, C2:])

            # PE p-state warmup on zeroed scratch while dma0 is in flight
            wu_l = sml.tile([128, 2, 128], fp8)
            nc.gpsimd.memset(wu_l[:], 0.0)
            wu_r = sml.tile([128, 2, 512], fp8)
            nc.gpsimd.memset(wu_r[:], 0.0)
            for i in range(14):
                wps = psum.tile([128, 512], f32, tag="ps2", bufs=3,
                                name=f"wu_{i}")
                nc.tensor.matmul(wps[:], wu_l[:], wu_r[:],
                                 start=True, stop=True, perf_mode=DR)

            ms_v = ax_sb[:, 0:NU * nv].rearrange("p (u v) -> p u v", u=NU)
            x0 = NU * nv + 8
            ix_w = []
            for u in range(NU):
                ixt = sml.tile([128, nvi], u16, name=f"ix_{u}")
                nc.vector.tensor_copy(
                    ixt[:],
                    ax_sb[:, x0 + u * nvi:x0 + (u + 1) * nvi].bitcast(u16))
                ix_w.append(ixt)
            sg_sb = sml.tile([128, 8], bf16)
            nc.vector.tensor_copy(sg_sb[:], ax_sb[:, NU * nv:NU * nv + 8])

            # S units: unit u = (q = u>>2, mt = u&3), cols
            # [q*1024, q*1024+1024).  4 DR matmuls -> [128,1024] psum ->
            # one bf16 cast -> indirect_copy (f32-pair view) -> strided
            # one-hot mult; seg-sum + sigmoid per unit-pair.
            o_sb = sml.tile([8, NU * nv], f32)
            p_sb = big.tile([128, NU, nv], bf16)
            for u in range(NU):
                q, mt = u >> 2, u & 3
                c0 = q * UCOL
                sw = big.tile([128, UCOL], bf16, name=f"sw_{u}", tag="sw",
                              bufs=4)
                ps = psum.tile([128, UCOL], f32, tag="ps2", bufs=3,
                               name=f"s_{u}")
                for nch in range(2):
                    for jc2 in range(2):
                        nc.tensor.matmul(
                            ps[:, nch * 512:(nch + 1) * 512],
                            pk_sb[:, 2 * jc2:2 * jc2 + 2,
                                  mt * 128:(mt + 1) * 128],
                            pk_sb[:, 2 * jc2:2 * jc2 + 2,
                                  BLK + c0 + nch * 512:
                                  BLK + c0 + (nch + 1) * 512],
                            start=(jc2 == 0), stop=(jc2 == 1), perf_mode=DR)
                if u % 2 == 0 or u < 2:
                    nc.scalar.copy(sw[:], ps[:])
                else:
                    nc.vector.tensor_copy(sw[:], ps[:])
                g_w = big.tile([128, nv], f32, name=f"g_{u}")
                nc.gpsimd.indirect_copy(g_w[:], sw[:].bitcast(f32),
                                        ix_w[u][:],
                                        i_know_ap_gather_is_preferred=True)
                gbf = g_w[:].bitcast(bf16).rearrange(
                    "p (v two) -> p v two", two=2)
                nc.vector.tensor_tensor(p_sb[:, u, 0:nv2],
                                        gbf[:, 0:nv2, 0],
                                        ms_v[:, u, 0:nv2],
                                        op=mybir.AluOpType.mult)
                nc.vector.tensor_tensor(p_sb[:, u, nv2:nv],
                                        gbf[:, nv2:nv, 1],
                                        ms_v[:, u, nv2:nv],
                                        op=mybir.AluOpType.mult)
                if u % 2 == 1:
                    pru = p_sb[:, u - 1:u + 1, :].rearrange(
                        "p u v -> p (u v)")
                    base = (u - 1) * nv
                    for cc in range(0, 2 * nv, 512):
                        ce = min(cc + 512, 2 * nv)
                        pr = psum.tile([8, ce - cc], f32, tag="seg",
                                       name=f"pr_{u}_{cc}", bufs=2)
                        nc.tensor.matmul(pr[:], sg_sb[:], pru[:, cc:ce],
                                         start=True, stop=True)
                        nc.scalar.activation(
                            o_sb[:, base + cc:base + ce], pr[:],
                            mybir.ActivationFunctionType.Sigmoid,
                            scale=1.0 / (SU * SU * SU))
            nc.sync.dma_start(OUT.ap(), o_sb[:])

    nc.compile()
    return nc


def _get_program(nv):
    if nv not in _cache:
        _cache[nv] = _build(nv)
    return _cache[nv]


def kernel(z_drug, global_weight, local_diag, batch_edges, edge_sub_type_idx,
           **_unused):
    from concourse.bass_utils import run_bass_kernel_spmd

    z = np.asarray(z_drug, np.float32)
    W = np.asarray(global_weight, np.float32)
    ld = np.asarray(local_diag, np.float32)
    e = np.asarray(batch_edges)
    sub = int(np.asarray(edge_sub_type_idx))
    d = ld[sub]
    assert z.shape == (N_DRUGS, D) and W.shape == (D, D)
    B = e.shape[1]
    e0 = e[0].astype(np.int64)
    e1 = e[1].astype(np.int64)

    U = z * d                                           # [4096, 512] f32
    A = U @ W                                           # [4096, 512] f32
    zT8 = np.ascontiguousarray((U.T * SU)).astype(F8)   # [512, 4096] fp8

    core = e0 // BLK
    r = e0 - core * BLK
    n = e1
    w = (n >> 10) * 4 + (r >> 7)                        # extraction unit
    g = (r & 127) >> 4                                  # 16-partition group
    par = (n & 1).astype(np.int64)                      # column parity
    lo = r & 15
    idx = (n & 1023) >> 1                               # f32-pair index

    # slot i within each (core, unit, group, parity) bucket
    order = np.lexsort((np.arange(B), par, g, w, core))
    cs = core[order]
    key = (((core * NU + w) * 8 + g) * 2 + par)[order]
    nb = N_CORES * NU * 8 * 2
    start = np.searchsorted(key, np.arange(nb), side="left")
    counts = np.bincount(key, minlength=nb)
    slot = np.arange(B) - start[key]
    nv2 = max(16, int(-(-counts.max() // 16)) * 16)
    nv = 2 * nv2
    nvi = nv // 16

    seg = np.zeros((128, 8), BF)
    for gg in range(8):
        seg[16 * gg:16 * gg + 16, gg] = BF(1.0)

    in_maps = []
    positions = []
    for c in range(N_CORES):
        m = order[cs == c]
        wc, gc = w[m], g[m]
        ic = slot[cs == c] + par[m] * nv2               # slot within unit
        ix = np.zeros((128, NU, nvi), np.uint16)
        ix[16 * gc + ic % 16, wc, ic // 16] = idx[m].astype(np.uint16)
        mask = np.zeros((128, NU, nv), BF)
        mask[16 * gc + lo[m], wc, ic] = BF(1.0)
        a8T = np.ascontiguousarray(
            (A[c * BLK:(c + 1) * BLK].T * (SU * SU))).astype(F8)
        pack = np.concatenate([a8T, zT8], axis=1)       # [512, PKW] fp8
        aux = np.concatenate(
            [mask.reshape(128, -1),
             seg,
             ix.reshape(128, -1).view(BF)], axis=1)     # [128, AXW] bf16
        in_maps.append({"pack": pack, "aux": aux})
        positions.append((m, gc, wc * nv + ic))

    nc = _get_program(nv)
    res = run_bass_kernel_spmd(nc, in_maps, list(range(N_CORES)))

    out = np.empty(B, np.float32)
    for c in range(N_CORES):
        oc = np.asarray(res.results[c]["out"], np.float32)  # [8, NU*nv]
        m, gc, col = positions[c]
        out[m] = oc[gc, col]
    return out


if __name__ == "__main__":
    dat = np.load("/root/problem/cached_io.npz")
    inputs = {k: dat[k] for k in ("z_drug", "global_weight", "local_diag",
                                  "batch_edges", "edge_sub_type_idx")}
    expected = dat["expected"]
    actual = kernel(**inputs)
    err = np.abs(actual - expected)
    print("max abs err:", err.max(), "mean:", err.mean())
    print("Relative error:", err.max() / np.abs(expected).max())


# revision 23
# speedup vs baseline: 3.4374x; 1.0212x over previous
"""Trainium2 Bass kernel for the Dedicom decoder problem.

Math: with U = z * d (row-wise scale by the selected local_diag row),
    score_b = ((z[e0]*d) @ W) * d . z[e1] = U[e0] @ W @ U[e1]^T
so all-pairs scores S = A @ U^T with A = U @ W contain every edge score.
A is edge-independent, so the host precomputes it (f32) and ships
A^T x256 and U^T x16 in fp8 (scores only span |S| < 0.5, so fp8 keeps
the sigmoid error ~1e-3).  Core c computes its 512-row block of S as 64
DoubleRow matmuls in 16 column-units of [128 rows, 1024 cols], drained
PSUM -> SBUF bf16 by one cast per unit (alternating DVE/ACT).
Per-edge extraction runs on the otherwise-idle GPSIMD engine via
indirect_copy over the unit's data BITCAST TO F32 (so the scan is 512
pair-elements, halving GPSIMD time); each fetched f32 is a pair of
scores and the index lists are parity-sorted so a strided one-hot
multiply (DVE) picks the right half.  The 16-way partition redundancy
of indirect_copy is resolved by that same host mask + a PE segment-sum
(lhsT = 16-partition segment indicator), then sigmoid(x/4096) on ACT.
All inputs arrive in 4 packed DMAs (HWDGE issue cost dominates small
transfers); index lists are unpacked on-chip; dummy matmuls warm the PE
p-state during the first DMA.  Edges are bucketed on the host by
(core, unit, group, column-parity); results are unscattered on the
host.
"""

import numpy as np
import ml_dtypes

BF = ml_dtypes.bfloat16
F8 = ml_dtypes.float8_e4m3fn

N_DRUGS = 4096
D = 512
N_CORES = 8
BLK = N_DRUGS // N_CORES  # 512 rows of S per core
KC = D // 128             # 4 contraction chunks
MT = BLK // 128           # 4 row tiles of the core's S block
NU = 16                   # extraction units: (col-quarter, row-tile)
UCOL = 1024               # columns per unit
SU = 16.0                 # host pre-scale on U and W (fp8 dynamic range)
PKW = BLK + N_DRUGS       # packed matrix cols: a8T | zt

_cache = {}


def _build(nv):
    """Build + compile the SPMD program; `nv` = 2*nv2 slots per
    (unit, group): even-parity slots then odd-parity slots."""
    import concourse.bass as bass  # noqa: F401
    import concourse.bacc as bacc
    import concourse.mybir as mybir
    import concourse.tile as tile

    f32 = mybir.dt.float32
    bf16 = mybir.dt.bfloat16
    fp8 = mybir.dt.float8e4
    u16 = mybir.dt.uint16
    DR = mybir.MatmulPerfMode.DoubleRow

    nv2 = nv // 2
    nvi = nv // 16
    # aux pack (bf16 cols): mask [NU*nv] | seg [8] | idx-as-bf16 [NU*nvi]
    AXW = NU * nv + 8 + NU * nvi

    nc = bacc.Bacc("TRN2", target_bir_lowering=False, debug=False,
                   num_devices=N_CORES)

    PK = nc.dram_tensor("pack", [D, PKW], fp8, kind="ExternalInput")
    AX = nc.dram_tensor("aux", [128, AXW], bf16, kind="ExternalInput")
    OUT = nc.dram_tensor("out", [8, NU * nv], f32, kind="ExternalOutput")

    with tile.TileContext(nc) as tc:
        with (
            tc.tile_pool(name="big", bufs=1) as big,
            tc.tile_pool(name="sml", bufs=1) as sml,
            tc.tile_pool(name="psum", bufs=8, space="PSUM") as psum,
        ):
            pk_sb = big.tile([128, KC, PKW], fp8)
            pk_v = PK.ap().rearrange("(kc p) n -> p kc n", p=128)
            C1 = BLK + UCOL        # dma0: a8T + zt quarter 0
            C2 = BLK + 3 * UCOL    # dma1: zt quarters 1-2
            nc.sync.dma_start(pk_sb[:, :, 0:C1], pk_v[:, :, 0:C1])
            ax_sb = big.tile([128, AXW], bf16)
            nc.scalar.dma_start(ax_sb[:], AX.ap())
            nc.sync.dma_start(pk_sb[:, :, C1:C2], pk_v[:, :, C1:C2])
            nc.sync.dma_start(pk_sb[:, :, C2:], pk_v[:, :, C2:])

            # PE p-state warmup on zeroed scratch while dma0 is in flight
            wu_l = sml.tile([128, 2, 128], fp8)
            nc.gpsimd.memset(wu_l[:], 0.0)
            wu_r = sml.tile([128, 2, 512], fp8)
            nc.gpsimd.memset(wu_r[:], 0.0)
            # trigger the sigmoid act-table load during the lead-in (the
            # sigmoid set also serves Copy, so later casts need no reload)
            dum = sml.tile([128, 2], f32)
            nc.scalar.activation(dum[:], wu_r[:, 0, 0:2],
                                 mybir.ActivationFunctionType.Sigmoid)
            for i in range(14):
                wps = psum.tile([128, 512], f32, tag="ps2", bufs=3,
                                name=f"wu_{i}")
                nc.tensor.matmul(wps[:], wu_l[:], wu_r[:],
                                 start=True, stop=True, perf_mode=DR)

            ms_v = ax_sb[:, 0:NU * nv].rearrange("p (u v) -> p u v", u=NU)
            x0 = NU * nv + 8
            ix_w = []
            for u in range(NU):
                ixt = sml.tile([128, nvi], u16, name=f"ix_{u}")
                nc.vector.tensor_copy(
                    ixt[:],
                    ax_sb[:, x0 + u * nvi:x0 + (u + 1) * nvi].bitcast(u16))
                ix_w.append(ixt)
            sg_sb = sml.tile([128, 8], bf16)
            nc.vector.tensor_copy(sg_sb[:], ax_sb[:, NU * nv:NU * nv + 8])

            # S units: unit u = (q = u>>2, mt = u&3), cols
            # [q*1024, q*1024+1024).  4 DR matmuls -> [128,1024] psum ->
            # one bf16 cast -> indirect_copy (f32-pair view) -> strided
            # one-hot mult; seg-sum + sigmoid per unit-pair.
            o_sb = sml.tile([8, NU * nv], f32)
            p_sb = big.tile([128, NU, nv], bf16)
            for u in range(NU):
                q, mt = u >> 2, u & 3
                c0 = q * UCOL
                sw = big.tile([128, UCOL], bf16, name=f"sw_{u}", tag="sw",
                              bufs=4)
                ps = psum.tile([128, UCOL], f32, tag="ps2", bufs=3,
                               name=f"s_{u}")
                for nch in range(2):
                    for jc2 in range(2):
                        nc.tensor.matmul(
                            ps[:, nch * 512:(nch + 1) * 512],
                            pk_sb[:, 2 * jc2:2 * jc2 + 2,
                                  mt * 128:(mt + 1) * 128],
                            pk_sb[:, 2 * jc2:2 * jc2 + 2,
                                  BLK + c0 + nch * 512:
                                  BLK + c0 + (nch + 1) * 512],
                            start=(jc2 == 0), stop=(jc2 == 1), perf_mode=DR)
                if u % 2 == 0 or u < 2:
                    nc.scalar.copy(sw[:], ps[:])
                else:
                    nc.vector.tensor_copy(sw[:], ps[:])
                g_w = big.tile([128, nv], f32, name=f"g_{u}")
                nc.gpsimd.indirect_copy(g_w[:], sw[:].bitcast(f32),
                                        ix_w[u][:],
                                        i_know_ap_gather_is_preferred=True)
                gbf = g_w[:].bitcast(bf16).rearrange(
                    "p (v two) -> p v two", two=2)
                nc.vector.tensor_tensor(p_sb[:, u, 0:nv2],
                                        gbf[:, 0:nv2, 0],
                                        ms_v[:, u, 0:nv2],
                                        op=mybir.AluOpType.mult)
                nc.vector.tensor_tensor(p_sb[:, u, nv2:nv],
                                        gbf[:, nv2:nv, 1],
                                        ms_v[:, u, nv2:nv],
                                        op=mybir.AluOpType.mult)
                if u % 2 == 1:
                    pru = p_sb[:, u - 1:u + 1, :].rearrange(
                        "p u v -> p (u v)")
                    base = (u - 1) * nv
                    for cc in range(0, 2 * nv, 512):
                        ce = min(cc + 512, 2 * nv)
                        pr = psum.tile([8, ce - cc], f32, tag="seg",
                                       name=f"pr_{u}_{cc}", bufs=2)
                        nc.tensor.matmul(pr[:], sg_sb[:], pru[:, cc:ce],
                                         start=True, stop=True)
                        nc.scalar.activation(
                            o_sb[:, base + cc:base + ce], pr[:],
                            mybir.ActivationFunctionType.Sigmoid,
                            scale=1.0 / (SU * SU * SU))
                if u == 11:
                    nc.sync.dma_start(OUT.ap()[:, 0:12 * nv],
                                      o_sb[:, 0:12 * nv])
            nc.sync.dma_start(OUT.ap()[:, 12 * nv:], o_sb[:, 12 * nv:])

    nc.compile()
    return nc


def _get_program(nv):
    if nv not in _cache:
        _cache[nv] = _build(nv)
    return _cache[nv]


def kernel(z_drug, global_weight, local_diag, batch_edges, edge_sub_type_idx,
           **_unused):
    from concourse.bass_utils import run_bass_kernel_spmd

    z = np.asarray(z_drug, np.float32)
    W = np.asarray(global_weight, np.float32)
    ld = np.asarray(local_diag, np.float32)
    e = np.asarray(batch_edges)
    sub = int(np.asarray(edge_sub_type_idx))
    d = ld[sub]
    assert z.shape == (N_DRUGS, D) and W.shape == (D, D)
    B = e.shape[1]
    e0 = e[0].astype(np.int64)
    e1 = e[1].astype(np.int64)

    U = z * d                                           # [4096, 512] f32
    A = U @ W                                           # [4096, 512] f32
    zT8 = np.ascontiguousarray((U.T * SU)).astype(F8)   # [512, 4096] fp8

    core = e0 // BLK
    r = e0 - core * BLK
    n = e1
    w = (n >> 10) * 4 + (r >> 7)                        # extraction unit
    g = (r & 127) >> 4                                  # 16-partition group
    par = (n & 1).astype(np.int64)                      # column parity
    lo = r & 15
    idx = (n & 1023) >> 1                               # f32-pair index

    # slot i within each (core, unit, group, parity) bucket
    order = np.lexsort((np.arange(B), par, g, w, core))
    cs = core[order]
    key = (((core * NU + w) * 8 + g) * 2 + par)[order]
    nb = N_CORES * NU * 8 * 2
    start = np.searchsorted(key, np.arange(nb), side="left")
    counts = np.bincount(key, minlength=nb)
    slot = np.arange(B) - start[key]
    nv2 = max(16, int(-(-counts.max() // 16)) * 16)
    nv = 2 * nv2
    nvi = nv // 16

    seg = np.zeros((128, 8), BF)
    for gg in range(8):
        seg[16 * gg:16 * gg + 16, gg] = BF(1.0)

    in_maps = []
    positions = []
    for c in range(N_CORES):
        m = order[cs == c]
        wc, gc = w[m], g[m]
        ic = slot[cs == c] + par[m] * nv2               # slot within unit
        ix = np.zeros((128, NU, nvi), np.uint16)
        ix[16 * gc + ic % 16, wc, ic // 16] = idx[m].astype(np.uint16)
        mask = np.zeros((128, NU, nv), BF)
        mask[16 * gc + lo[m], wc, ic] = BF(1.0)
        a8T = np.ascontiguousarray(
            (A[c * BLK:(c + 1) * BLK].T * (SU * SU))).astype(F8)
        pack = np.concatenate([a8T, zT8], axis=1)       # [512, PKW] fp8
        aux = np.concatenate(
            [mask.reshape(128, -1),
             seg,
             ix.reshape(128, -1).view(BF)], axis=1)     # [128, AXW] bf16
        in_maps.append({"pack": pack, "aux": aux})
        positions.append((m, gc, wc * nv + ic))

    nc = _get_program(nv)
    res = run_bass_kernel_spmd(nc, in_maps, list(range(N_CORES)))

    out = np.empty(B, np.float32)
    for c in range(N_CORES):
        oc = np.asarray(res.results[c]["out"], np.float32)  # [8, NU*nv]
        m, gc, col = positions[c]
        out[m] = oc[gc, col]
    return out


if __name__ == "__main__":
    dat = np.load("/root/problem/cached_io.npz")
    inputs = {k: dat[k] for k in ("z_drug", "global_weight", "local_diag",
                                  "batch_edges", "edge_sub_type_idx")}
    expected = dat["expected"]
    actual = kernel(**inputs)
    err = np.abs(actual - expected)
    print("max abs err:", err.max(), "mean:", err.mean())
    print("Relative error:", err.max() / np.abs(expected).max())
